# revision 19
# baseline (speedup 1.0000x reference)
"""Multi-head causal attention (B=2, S=2048, E=1024, H=16, D=64) on 8 TRN2 cores.

Sharding: core c handles batch b = c//4 and head-group g = c%4 (4 heads,
256 projection columns). Each core computes its partial out-projection
(ctx @ W_out[rows]); the host sums the 4 partials per batch.

Per-core pipeline (single SPMD program, bf16 matmuls / fp32 accumulation):
  1. X -> bf16 -> X^T [e, s] via DMA xbar transpose (128x128 tiles)
  2. Q^T, K^T bf16 [n, s] (q pre-scaled by 1/sqrt(D)), V bf16 [s, (h, d+1)]
     with a ones column per head for the softmax denominator
  3. per (head-pair, q-tile of 512): S^T = K Q^T with both heads packed
     into disjoint PE row-groups (d=64 each), exp on ACT -> P^T bf16,
     causal mask via gpsimd affine_select, ctx^T = [V|1].T @ P^T
     accumulated in PSUM; row D = denominator; normalize via
     outer-product broadcast of 1/den (fp16 ones x recip matmul).
  4. out = ctx @ W_out slice (bf16), DMA out.
"""

import sys

if "/opt/trn_rl_repo" not in sys.path:
    sys.path.insert(0, "/opt/trn_rl_repo")

from contextlib import ExitStack

import numpy as np

import concourse.bacc as bacc
import concourse.mybir as mybir
import concourse.tile as tile
from concourse.bass_utils import run_bass_kernel_spmd

P = 128
S = 2048
E = 1024
HC = 256          # head columns per core (4 heads x 64)
D = 64
NHC = 4           # heads per core
SC = S // P       # 16 s-chunks
ECH = E // P      # 8 e-chunks
QT = 512          # q-tile
NQT = S // QT     # 4

F32 = mybir.dt.float32
BF16 = mybir.dt.bfloat16
FP16 = mybir.dt.float16
EXP = mybir.ActivationFunctionType.Exp


def build_nc():
    nc = bacc.Bacc("TRN2", target_bir_lowering=False)
    x = nc.dram_tensor("x", [S, E], BF16, kind="ExternalInput")
    wq = nc.dram_tensor("wq", [E, HC], BF16, kind="ExternalInput")
    wk = nc.dram_tensor("wk", [E, HC], BF16, kind="ExternalInput")
    wv = nc.dram_tensor("wv", [E, HC], BF16, kind="ExternalInput")
    wo = nc.dram_tensor("wo", [HC, E], BF16, kind="ExternalInput")
    out = nc.dram_tensor("out", [S, E], F32, kind="ExternalOutput")

    with tile.TileContext(nc) as tc, ExitStack() as ctx:
        sb = ctx.enter_context(tc.tile_pool(name="sb", bufs=1))
        stage = ctx.enter_context(tc.tile_pool(name="stage", bufs=3))
        ps = ctx.enter_context(tc.tile_pool(name="ps", bufs=1, space="PSUM"))

        # ---- weights: direct bf16 DMA ----
        wq_sb = sb.tile([P, ECH, HC], BF16)
        wk_sb = sb.tile([P, ECH, HC], BF16)
        wv_sb = sb.tile([P, ECH, HC], BF16)
        wo_sb = sb.tile([P, 2, E], BF16)
        for w_dram, w_bf, pat in (
            (wq, wq_sb, "(eo p) n -> p eo n"),
            (wk, wk_sb, "(eo p) n -> p eo n"),
            (wv, wv_sb, "(eo p) n -> p eo n"),
            (wo, wo_sb, "(c p) m -> p c m"),
        ):
            nc.sync.dma_start(w_bf[:], w_dram.rearrange(pat, p=P))

        # ---- phase 1: X^T by DMA xbar transpose straight from DRAM ----
        xT = sb.tile([P, ECH, S], BF16)
        for sh in range(2):
            s_sl = slice(sh * (S // 2), (sh + 1) * (S // 2))
            for ec in range(ECH):
                nc.sync.dma_start(
                    xT[:, ec, s_sl],
                    x[s_sl, ec * P : (ec + 1) * P],
                    transpose=True,
                )

        # ---- phase 2: projections (bf16 matmuls, fp32 psum) ----
        qT = sb.tile([P, 2, S], BF16)
        kT = sb.tile([P, 2, S], BF16)
        vC = sb.tile([P, SC, NHC, 2 * D], BF16)
        nc.gpsimd.memset(vC[:, :, :, D : 2 * D], 1.0)
        for st in range(NQT):
            for hp in range(2):
                pq = ps.tile([P, QT], F32, tag="mm", bufs=1)
                for ec in range(ECH):
                    nc.tensor.matmul(
                        pq[:],
                        wq_sb[:, ec, hp * P : (hp + 1) * P],
                        xT[:, ec, st * QT : (st + 1) * QT],
                        start=(ec == 0),
                        stop=(ec == ECH - 1),
                    )
                nc.vector.tensor_scalar_mul(
                    qT[:, hp, st * QT : (st + 1) * QT], pq[:], 0.125
                )
                pk = ps.tile([P, QT], F32, tag="mm", bufs=1)
                for ec in range(ECH):
                    nc.tensor.matmul(
                        pk[:],
                        wk_sb[:, ec, hp * P : (hp + 1) * P],
                        xT[:, ec, st * QT : (st + 1) * QT],
                        start=(ec == 0),
                        stop=(ec == ECH - 1),
                    )
                nc.vector.tensor_copy(kT[:, hp, st * QT : (st + 1) * QT], pk[:])
            for sc in range(4 * st, 4 * st + 4):
                pv = ps.tile([P, HC], F32, tag="mm", bufs=1)
                for ec in range(ECH):
                    nc.tensor.matmul(
                        pv[:],
                        xT[:, ec, sc * P : (sc + 1) * P],
                        wv_sb[:, ec, :],
                        start=(ec == 0),
                        stop=(ec == ECH - 1),
                    )
                for h in range(NHC):
                    nc.vector.tensor_copy(
                        vC[:, sc, h, 0:D], pv[:, h * D : (h + 1) * D]
                    )

        # ---- phase 3: attention, head PAIRS packed in PE row groups ----
        # vC carries 64 ones-columns, so rows D..2D-1 of each ctx psum hold
        # the softmax denominator pre-broadcast; normalization is a pure
        # DVE chain (copy, reciprocal, multiply) that never stalls PE.
        ctxT = sb.tile([P, 2, S], BF16)
        for hp in range(2):
            hA, hB = 2 * hp, 2 * hp + 1
            for qt in range(NQT):
                nkb = 4 * (qt + 1)
                cpsA = ps.tile([P, QT], F32, tag="ctx", bufs=3)
                cpsB = ps.tile([P, QT], F32, tag="ctx", bufs=3)
                q_sl = slice(qt * QT, (qt + 1) * QT)
                pts = {}

                def emit_scores(kb):
                    stp = ps.tile([P, 2, QT], F32, tag="st", bufs=2)
                    k_sl = slice(kb * P, (kb + 1) * P)
                    vq0 = max(0, kb * P - qt * QT)  # first valid q column
                    nc.tensor.matmul(
                        stp[:, 0, vq0:QT], kT[0:D, hp, k_sl],
                        qT[0:D, hp, qt * QT + vq0 : (qt + 1) * QT],
                        start=True, stop=True,
                    )
                    nc.tensor.matmul(
                        stp[:, 1, vq0:QT], kT[D:P, hp, k_sl],
                        qT[D:P, hp, qt * QT + vq0 : (qt + 1) * QT],
                        start=True, stop=True,
                    )
                    pt = stage.tile([P, 2, QT], BF16, tag="pt", bufs=8)
                    nc.scalar.activation(pt[:, :, vq0:QT], stp[:, :, vq0:QT], EXP)
                    if vq0 > 0 or kb * P >= qt * QT:
                        # local 128-wide triangle at the diagonal
                        for half in range(2):
                            nc.gpsimd.affine_select(
                                out=pt[:, half, vq0 : vq0 + P],
                                in_=pt[:, half, vq0 : vq0 + P],
                                compare_op=mybir.AluOpType.is_ge,
                                fill=0.0,
                                base=0,
                                channel_multiplier=-1,
                                pattern=[[1, P]],
                            )
                    pts[kb] = (pt, vq0)

                def emit_ctx(kb):
                    pt, vq0 = pts.pop(kb)
                    nc.tensor.matmul(
                        cpsA[:, vq0:QT], vC[:, kb, hA, :], pt[:, 0, vq0:QT],
                        start=(kb == 0), stop=(kb == nkb - 1),
                        skip_group_check=True,
                    )
                    nc.tensor.matmul(
                        cpsB[:, vq0:QT], vC[:, kb, hB, :], pt[:, 1, vq0:QT],
                        start=(kb == 0), stop=(kb == nkb - 1),
                        skip_group_check=True,
                    )

                SKEW = min(6, nkb - 1)
                for kb in range(nkb):
                    emit_scores(kb)
                    if kb >= SKEW:
                        emit_ctx(kb - SKEW)
                for kb in range(nkb - SKEW, nkb):
                    emit_ctx(kb)
                for cps, sub in ((cpsA, 0), (cpsB, D)):
                    cuV = stage.tile([D, QT], F32, tag="cuV", bufs=4)
                    nc.vector.tensor_copy(cuV[:], cps[0:D, :])
                    cuD = stage.tile([D, QT], F32, tag="cuD", bufs=4)
                    nc.vector.reciprocal(cuD[:], cps[D:P, :])
                    nc.vector.tensor_mul(
                        ctxT[sub : sub + D, hp, q_sl], cuV[:], cuD[:]
                    )

        # ---- phase 4: out-projection (partial; host sums over cores) ----
        for sc in range(SC):
            for mh in range(2):
                po = ps.tile([P, QT], F32, tag="mm", bufs=1)
                for c2 in range(2):
                    nc.tensor.matmul(
                        po[:],
                        ctxT[:, c2, sc * P : (sc + 1) * P],
                        wo_sb[:, c2, mh * QT : (mh + 1) * QT],
                        start=(c2 == 0),
                        stop=(c2 == 1),
                    )
                ob = stage.tile([P, QT], F32, tag="ob", bufs=3)
                nc.scalar.copy(ob[:], po[:])
                nc.sync.dma_start(
                    out[sc * P : (sc + 1) * P, mh * QT : (mh + 1) * QT], ob[:]
                )

    nc.finalize()
    return nc


_NC = None


def _get_nc():
    global _NC
    if _NC is None:
        _NC = build_nc()
    return _NC


def _bf16(a):
    import ml_dtypes

    return np.ascontiguousarray(np.asarray(a, dtype=np.float32)).astype(
        ml_dtypes.bfloat16
    )


def make_in_maps(X_emb, W_q, W_k, W_v, W_out):
    in_maps = []
    for c in range(8):
        b, g = c // 4, c % 4
        cols = slice(g * HC, (g + 1) * HC)
        in_maps.append(
            {
                "x": _bf16(X_emb[b]),
                "wq": _bf16(W_q[:, cols]),
                "wk": _bf16(W_k[:, cols]),
                "wv": _bf16(W_v[:, cols]),
                "wo": _bf16(W_out[cols, :]),
            }
        )
    return in_maps


def combine_outputs(outs):
    return np.stack(
        [
            outs[0] + outs[1] + outs[2] + outs[3],
            outs[4] + outs[5] + outs[6] + outs[7],
        ]
    ).astype(np.float32)


def kernel(X_emb, W_q, W_k, W_v, W_out, _spmd_kwargs=None):
    X_emb = np.asarray(X_emb, dtype=np.float32)
    nc = _get_nc()
    in_maps = make_in_maps(X_emb, W_q, W_k, W_v, W_out)
    res = run_bass_kernel_spmd(
        nc, in_maps, core_ids=list(range(8)), **(_spmd_kwargs or {})
    )
    outs = [res.results[c]["out"] for c in range(8)]
    full = combine_outputs(outs)
    if _spmd_kwargs:
        kernel.last_result = res
    return full


# revision 20
# speedup vs baseline: 1.1105x; 1.1105x over previous
"""Multi-head causal attention (B=2, S=2048, E=1024, H=16, D=64) on 8 TRN2 cores.

Sharding: core c handles batch b = c//4 and head-group g = c%4 (4 heads,
256 projection columns). Each core computes its partial out-projection
(ctx @ W_out[rows]); the host sums the 4 partials per batch.

Per-core pipeline (single SPMD program, bf16 matmuls / fp32 accumulation):
  1. X -> bf16 -> X^T [e, s] via DMA xbar transpose (128x128 tiles)
  2. Q^T, K^T bf16 [n, s] (q pre-scaled by 1/sqrt(D)), V bf16 [s, (h, d+1)]
     with a ones column per head for the softmax denominator
  3. per (head-pair, q-tile of 512): S^T = K Q^T with both heads packed
     into disjoint PE row-groups (d=64 each), exp on ACT -> P^T bf16,
     causal mask via gpsimd affine_select, ctx^T = [V|1].T @ P^T
     accumulated in PSUM; row D = denominator; normalize via
     outer-product broadcast of 1/den (fp16 ones x recip matmul).
  4. out = ctx @ W_out slice (bf16), DMA out.
"""

import sys

if "/opt/trn_rl_repo" not in sys.path:
    sys.path.insert(0, "/opt/trn_rl_repo")

from contextlib import ExitStack

import numpy as np

import concourse.bacc as bacc
import concourse.mybir as mybir
import concourse.tile as tile
from concourse.bass_utils import run_bass_kernel_spmd

P = 128
S = 2048
E = 1024
HC = 256          # head columns per core (4 heads x 64)
D = 64
NHC = 4           # heads per core
SC = S // P       # 16 s-chunks
ECH = E // P      # 8 e-chunks
QT = 512          # q-tile
NQT = S // QT     # 4

F32 = mybir.dt.float32
BF16 = mybir.dt.bfloat16
FP16 = mybir.dt.float16
EXP = mybir.ActivationFunctionType.Exp


def build_nc():
    nc = bacc.Bacc("TRN2", target_bir_lowering=False)
    x = nc.dram_tensor("x", [S, E], BF16, kind="ExternalInput")
    wq = nc.dram_tensor("wq", [E, HC], BF16, kind="ExternalInput")
    wk = nc.dram_tensor("wk", [E, HC], BF16, kind="ExternalInput")
    wv = nc.dram_tensor("wv", [E, HC], BF16, kind="ExternalInput")
    wo = nc.dram_tensor("wo", [HC, E], BF16, kind="ExternalInput")
    out = nc.dram_tensor("out", [S, E], F32, kind="ExternalOutput")

    with tile.TileContext(nc) as tc, ExitStack() as ctx:
        sb = ctx.enter_context(tc.tile_pool(name="sb", bufs=1))
        stage = ctx.enter_context(tc.tile_pool(name="stage", bufs=3))
        ps = ctx.enter_context(tc.tile_pool(name="ps", bufs=1, space="PSUM"))

        # ---- weights: direct bf16 DMA ----
        wq_sb = sb.tile([P, ECH, HC], BF16)
        wk_sb = sb.tile([P, ECH, HC], BF16)
        wv_sb = sb.tile([P, ECH, HC], BF16)
        wo_sb = sb.tile([P, 2, E], BF16)
        for w_dram, w_bf, pat in (
            (wq, wq_sb, "(eo p) n -> p eo n"),
            (wk, wk_sb, "(eo p) n -> p eo n"),
            (wv, wv_sb, "(eo p) n -> p eo n"),
            (wo, wo_sb, "(c p) m -> p c m"),
        ):
            nc.sync.dma_start(w_bf[:], w_dram.rearrange(pat, p=P))

        # ---- phase 1: X^T by DMA xbar transpose straight from DRAM ----
        xT = sb.tile([P, ECH, S], BF16)
        for sh in range(2):
            s_sl = slice(sh * (S // 2), (sh + 1) * (S // 2))
            for ec in range(ECH):
                nc.sync.dma_start(
                    xT[:, ec, s_sl],
                    x[s_sl, ec * P : (ec + 1) * P],
                    transpose=True,
                )

        # ---- phase 2: projections (bf16 matmuls, fp32 psum) ----
        qT = sb.tile([P, 2, S], BF16)
        kT = sb.tile([P, 2, S], BF16)
        vC = sb.tile([P, SC, NHC, 2 * D], BF16)
        nc.gpsimd.memset(vC[:, :, :, D : 2 * D], 1.0)
        for st in range(NQT):
            for hp in range(2):
                pq = ps.tile([P, QT], F32, tag="mm", bufs=2)
                for ec in range(ECH):
                    nc.tensor.matmul(
                        pq[:],
                        wq_sb[:, ec, hp * P : (hp + 1) * P],
                        xT[:, ec, st * QT : (st + 1) * QT],
                        start=(ec == 0),
                        stop=(ec == ECH - 1),
                    )
                nc.vector.tensor_scalar_mul(
                    qT[:, hp, st * QT : (st + 1) * QT], pq[:], 0.125
                )
                pk = ps.tile([P, QT], F32, tag="mm", bufs=2)
                for ec in range(ECH):
                    nc.tensor.matmul(
                        pk[:],
                        wk_sb[:, ec, hp * P : (hp + 1) * P],
                        xT[:, ec, st * QT : (st + 1) * QT],
                        start=(ec == 0),
                        stop=(ec == ECH - 1),
                    )
                nc.vector.tensor_copy(kT[:, hp, st * QT : (st + 1) * QT], pk[:])
            for sc in range(4 * st, 4 * st + 4):
                pv = ps.tile([P, HC], F32, tag="mm", bufs=2)
                for ec in range(ECH):
                    nc.tensor.matmul(
                        pv[:],
                        xT[:, ec, sc * P : (sc + 1) * P],
                        wv_sb[:, ec, :],
                        start=(ec == 0),
                        stop=(ec == ECH - 1),
                    )
                for h in range(NHC):
                    nc.vector.tensor_copy(
                        vC[:, sc, h, 0:D], pv[:, h * D : (h + 1) * D]
                    )

        # ---- phase 3: attention, head PAIRS packed in PE row groups ----
        # vC carries 64 ones-columns, so rows D..2D-1 of each ctx psum hold
        # the softmax denominator pre-broadcast; normalization is a pure
        # DVE chain (copy, reciprocal, multiply) that never stalls PE.
        ctxT = sb.tile([P, 2, S], BF16)
        for hp in range(2):
            hA, hB = 2 * hp, 2 * hp + 1
            for qt in range(NQT):
                nkb = 4 * (qt + 1)
                cpsA = ps.tile([P, QT], F32, tag="ctx", bufs=2)
                cpsB = ps.tile([P, QT], F32, tag="ctx", bufs=2)
                q_sl = slice(qt * QT, (qt + 1) * QT)
                pts = {}

                def emit_scores(kb):
                    stp = ps.tile([P, 2, QT], F32, tag="st", bufs=2)
                    k_sl = slice(kb * P, (kb + 1) * P)
                    vq0 = max(0, kb * P - qt * QT)  # first valid q column
                    nc.tensor.matmul(
                        stp[:, 0, vq0:QT], kT[0:D, hp, k_sl],
                        qT[0:D, hp, qt * QT + vq0 : (qt + 1) * QT],
                        start=True, stop=True,
                    )
                    nc.tensor.matmul(
                        stp[:, 1, vq0:QT], kT[D:P, hp, k_sl],
                        qT[D:P, hp, qt * QT + vq0 : (qt + 1) * QT],
                        start=True, stop=True,
                    )
                    pt = stage.tile([P, 2, QT], BF16, tag="pt", bufs=8)
                    nc.scalar.activation(pt[:, :, vq0:QT], stp[:, :, vq0:QT], EXP)
                    if vq0 > 0 or kb * P >= qt * QT:
                        # local 128-wide triangle at the diagonal
                        for half in range(2):
                            nc.gpsimd.affine_select(
                                out=pt[:, half, vq0 : vq0 + P],
                                in_=pt[:, half, vq0 : vq0 + P],
                                compare_op=mybir.AluOpType.is_ge,
                                fill=0.0,
                                base=0,
                                channel_multiplier=-1,
                                pattern=[[1, P]],
                            )
                    pts[kb] = (pt, vq0)

                def emit_ctx(kb):
                    pt, vq0 = pts.pop(kb)
                    nc.tensor.matmul(
                        cpsA[:, vq0:QT], vC[:, kb, hA, :], pt[:, 0, vq0:QT],
                        start=(kb == 0), stop=(kb == nkb - 1),
                        skip_group_check=True,
                    )
                    nc.tensor.matmul(
                        cpsB[:, vq0:QT], vC[:, kb, hB, :], pt[:, 1, vq0:QT],
                        start=(kb == 0), stop=(kb == nkb - 1),
                        skip_group_check=True,
                    )

                SKEW = min(6, nkb - 1)
                for kb in range(nkb):
                    emit_scores(kb)
                    if kb >= SKEW:
                        emit_ctx(kb - SKEW)
                for kb in range(nkb - SKEW, nkb):
                    emit_ctx(kb)
                for cps, sub in ((cpsA, 0), (cpsB, D)):
                    cuV = stage.tile([D, QT], F32, tag="cuV", bufs=4)
                    nc.vector.tensor_copy(cuV[:], cps[0:D, :])
                    cuD = stage.tile([D, QT], F32, tag="cuD", bufs=4)
                    nc.vector.reciprocal(cuD[:], cps[D:P, :])
                    nc.vector.tensor_mul(
                        ctxT[sub : sub + D, hp, q_sl], cuV[:], cuD[:]
                    )

        # ---- phase 4: out-projection (partial; host sums over cores) ----
        for sc in range(SC):
            for mh in range(2):
                po = ps.tile([P, QT], F32, tag="mm", bufs=2)
                for c2 in range(2):
                    nc.tensor.matmul(
                        po[:],
                        ctxT[:, c2, sc * P : (sc + 1) * P],
                        wo_sb[:, c2, mh * QT : (mh + 1) * QT],
                        start=(c2 == 0),
                        stop=(c2 == 1),
                    )
                ob = stage.tile([P, QT], F32, tag="ob", bufs=3)
                nc.scalar.copy(ob[:], po[:])
                nc.sync.dma_start(
                    out[sc * P : (sc + 1) * P, mh * QT : (mh + 1) * QT], ob[:]
                )

    nc.finalize()
    return nc


_NC = None


def _get_nc():
    global _NC
    if _NC is None:
        _NC = build_nc()
    return _NC


def _bf16(a):
    import ml_dtypes

    return np.ascontiguousarray(np.asarray(a, dtype=np.float32)).astype(
        ml_dtypes.bfloat16
    )


def make_in_maps(X_emb, W_q, W_k, W_v, W_out):
    in_maps = []
    for c in range(8):
        b, g = c // 4, c % 4
        cols = slice(g * HC, (g + 1) * HC)
        in_maps.append(
            {
                "x": _bf16(X_emb[b]),
                "wq": _bf16(W_q[:, cols]),
                "wk": _bf16(W_k[:, cols]),
                "wv": _bf16(W_v[:, cols]),
                "wo": _bf16(W_out[cols, :]),
            }
        )
    return in_maps


def combine_outputs(outs):
    return np.stack(
        [
            outs[0] + outs[1] + outs[2] + outs[3],
            outs[4] + outs[5] + outs[6] + outs[7],
        ]
    ).astype(np.float32)


def kernel(X_emb, W_q, W_k, W_v, W_out, _spmd_kwargs=None):
    X_emb = np.asarray(X_emb, dtype=np.float32)
    nc = _get_nc()
    in_maps = make_in_maps(X_emb, W_q, W_k, W_v, W_out)
    res = run_bass_kernel_spmd(
        nc, in_maps, core_ids=list(range(8)), **(_spmd_kwargs or {})
    )
    outs = [res.results[c]["out"] for c in range(8)]
    full = combine_outputs(outs)
    if _spmd_kwargs:
        kernel.last_result = res
    return full


# revision 21
# speedup vs baseline: 1.1256x; 1.0136x over previous
"""Multi-head causal attention (B=2, S=2048, E=1024, H=16, D=64) on 8 TRN2 cores.

Sharding: core c handles batch b = c//4 and head-group g = c%4 (4 heads,
256 projection columns). Each core computes its partial out-projection
(ctx @ W_out[rows]); the host sums the 4 partials per batch.

Per-core pipeline (single SPMD program, bf16 matmuls / fp32 accumulation):
  1. X -> bf16 -> X^T [e, s] via DMA xbar transpose (128x128 tiles)
  2. Q^T, K^T bf16 [n, s] (q pre-scaled by 1/sqrt(D)), V bf16 [s, (h, d+1)]
     with a ones column per head for the softmax denominator
  3. per (head-pair, q-tile of 512): S^T = K Q^T with both heads packed
     into disjoint PE row-groups (d=64 each), exp on ACT -> P^T bf16,
     causal mask via gpsimd affine_select, ctx^T = [V|1].T @ P^T
     accumulated in PSUM; row D = denominator; normalize via
     outer-product broadcast of 1/den (fp16 ones x recip matmul).
  4. out = ctx @ W_out slice (bf16), DMA out.
"""

import sys

if "/opt/trn_rl_repo" not in sys.path:
    sys.path.insert(0, "/opt/trn_rl_repo")

from contextlib import ExitStack

import numpy as np

import concourse.bacc as bacc
import concourse.mybir as mybir
import concourse.tile as tile
from concourse.bass_utils import run_bass_kernel_spmd

P = 128
S = 2048
E = 1024
HC = 256          # head columns per core (4 heads x 64)
D = 64
NHC = 4           # heads per core
SC = S // P       # 16 s-chunks
ECH = E // P      # 8 e-chunks
QT = 512          # q-tile
NQT = S // QT     # 4

F32 = mybir.dt.float32
BF16 = mybir.dt.bfloat16
FP16 = mybir.dt.float16
EXP = mybir.ActivationFunctionType.Exp


def build_nc():
    nc = bacc.Bacc("TRN2", target_bir_lowering=False)
    x = nc.dram_tensor("x", [S, E], BF16, kind="ExternalInput")
    wq = nc.dram_tensor("wq", [E, HC], BF16, kind="ExternalInput")
    wk = nc.dram_tensor("wk", [E, HC], BF16, kind="ExternalInput")
    wv = nc.dram_tensor("wv", [E, HC], BF16, kind="ExternalInput")
    wo = nc.dram_tensor("wo", [HC, E], BF16, kind="ExternalInput")
    out = nc.dram_tensor("out", [S, E], F32, kind="ExternalOutput")

    with tile.TileContext(nc) as tc, ExitStack() as ctx:
        sb = ctx.enter_context(tc.tile_pool(name="sb", bufs=1))
        stage = ctx.enter_context(tc.tile_pool(name="stage", bufs=3))
        ps = ctx.enter_context(tc.tile_pool(name="ps", bufs=1, space="PSUM"))

        # ---- weights: direct bf16 DMA ----
        wq_sb = sb.tile([P, ECH, HC], BF16)
        wk_sb = sb.tile([P, ECH, HC], BF16)
        wv_sb = sb.tile([P, ECH, HC], BF16)
        wo_sb = sb.tile([P, 2, E], BF16)
        for w_dram, w_bf, pat in (
            (wq, wq_sb, "(eo p) n -> p eo n"),
            (wk, wk_sb, "(eo p) n -> p eo n"),
            (wv, wv_sb, "(eo p) n -> p eo n"),
            (wo, wo_sb, "(c p) m -> p c m"),
        ):
            nc.sync.dma_start(w_bf[:], w_dram.rearrange(pat, p=P))

        # ---- phase 1: X^T by DMA xbar transpose straight from DRAM ----
        xT = sb.tile([P, ECH, S], BF16)
        for sh in range(2):
            s_sl = slice(sh * (S // 2), (sh + 1) * (S // 2))
            for ec in range(ECH):
                nc.sync.dma_start(
                    xT[:, ec, s_sl],
                    x[s_sl, ec * P : (ec + 1) * P],
                    transpose=True,
                )

        # ---- phase 2: projections (bf16 matmuls, fp32 psum) ----
        qT = sb.tile([P, 2, S], BF16)
        kT = sb.tile([P, 2, S], BF16)
        vC = sb.tile([P, SC, NHC, 2 * D], BF16)
        nc.gpsimd.memset(vC[:, :, :, D : 2 * D], 1.0)
        for st in range(NQT):
            for hp in range(2):
                pq = ps.tile([P, QT], F32, tag="mm", bufs=2)
                for ec in range(ECH):
                    nc.tensor.matmul(
                        pq[:],
                        wq_sb[:, ec, hp * P : (hp + 1) * P],
                        xT[:, ec, st * QT : (st + 1) * QT],
                        start=(ec == 0),
                        stop=(ec == ECH - 1),
                    )
                nc.vector.tensor_scalar_mul(
                    qT[:, hp, st * QT : (st + 1) * QT], pq[:], 0.125
                )
                pk = ps.tile([P, QT], F32, tag="mm", bufs=2)
                for ec in range(ECH):
                    nc.tensor.matmul(
                        pk[:],
                        wk_sb[:, ec, hp * P : (hp + 1) * P],
                        xT[:, ec, st * QT : (st + 1) * QT],
                        start=(ec == 0),
                        stop=(ec == ECH - 1),
                    )
                nc.vector.tensor_copy(kT[:, hp, st * QT : (st + 1) * QT], pk[:])
            for sc in range(4 * st, 4 * st + 4):
                pv = ps.tile([P, HC], F32, tag="mm", bufs=2)
                for ec in range(ECH):
                    nc.tensor.matmul(
                        pv[:],
                        xT[:, ec, sc * P : (sc + 1) * P],
                        wv_sb[:, ec, :],
                        start=(ec == 0),
                        stop=(ec == ECH - 1),
                    )
                for h in range(NHC):
                    nc.vector.tensor_copy(
                        vC[:, sc, h, 0:D], pv[:, h * D : (h + 1) * D]
                    )

        # ---- phase 3: attention, head PAIRS packed in PE row groups ----
        # vC carries 64 ones-columns, so rows D..2D-1 of each ctx psum hold
        # the softmax denominator pre-broadcast; normalization is a pure
        # DVE chain (copy, reciprocal, multiply) that never stalls PE.
        ctxT = sb.tile([P, 2, S], BF16)
        for hp in range(2):
            hA, hB = 2 * hp, 2 * hp + 1
            for qt in range(NQT):
                nkb = 4 * (qt + 1)
                cpsA = ps.tile([P, QT], F32, tag="ctx", bufs=2)
                cpsB = ps.tile([P, QT], F32, tag="ctx", bufs=2)
                q_sl = slice(qt * QT, (qt + 1) * QT)
                pts = {}

                def emit_scores(kb):
                    stp = ps.tile([P, 2, QT], F32, tag="st", bufs=2)
                    k_sl = slice(kb * P, (kb + 1) * P)
                    vq0 = max(0, kb * P - qt * QT)  # first valid q column
                    nc.tensor.matmul(
                        stp[:, 0, vq0:QT], kT[0:D, hp, k_sl],
                        qT[0:D, hp, qt * QT + vq0 : (qt + 1) * QT],
                        start=True, stop=True,
                    )
                    nc.tensor.matmul(
                        stp[:, 1, vq0:QT], kT[D:P, hp, k_sl],
                        qT[D:P, hp, qt * QT + vq0 : (qt + 1) * QT],
                        start=True, stop=True,
                    )
                    pt = stage.tile([P, 2, QT], BF16, tag="pt", bufs=8)
                    nc.scalar.activation(pt[:, :, vq0:QT], stp[:, :, vq0:QT], EXP)
                    if vq0 > 0 or kb * P >= qt * QT:
                        # local 128-wide triangle at the diagonal
                        for half in range(2):
                            nc.gpsimd.affine_select(
                                out=pt[:, half, vq0 : vq0 + P],
                                in_=pt[:, half, vq0 : vq0 + P],
                                compare_op=mybir.AluOpType.is_ge,
                                fill=0.0,
                                base=0,
                                channel_multiplier=-1,
                                pattern=[[1, P]],
                            )
                    pts[kb] = (pt, vq0)

                def emit_ctx(kb):
                    pt, vq0 = pts.pop(kb)
                    nc.tensor.matmul(
                        cpsA[:, vq0:QT], vC[:, kb, hA, :], pt[:, 0, vq0:QT],
                        start=(kb == 0), stop=(kb == nkb - 1),
                        skip_group_check=True,
                    )
                    nc.tensor.matmul(
                        cpsB[:, vq0:QT], vC[:, kb, hB, :], pt[:, 1, vq0:QT],
                        start=(kb == 0), stop=(kb == nkb - 1),
                        skip_group_check=True,
                    )

                SKEW = min(4, nkb - 1)
                for kb in range(nkb):
                    emit_scores(kb)
                    if kb >= SKEW:
                        emit_ctx(kb - SKEW)
                for kb in range(nkb - SKEW, nkb):
                    emit_ctx(kb)
                for cps, sub in ((cpsA, 0), (cpsB, D)):
                    cuV = stage.tile([D, QT], F32, tag="cuV", bufs=4)
                    nc.vector.tensor_copy(cuV[:], cps[0:D, :])
                    cuD = stage.tile([D, QT], F32, tag="cuD", bufs=4)
                    nc.vector.reciprocal(cuD[:], cps[D:P, :])
                    nc.vector.tensor_mul(
                        ctxT[sub : sub + D, hp, q_sl], cuV[:], cuD[:]
                    )

        # ---- phase 4: out-projection (partial; host sums over cores) ----
        for sc in range(SC):
            for mh in range(2):
                po = ps.tile([P, QT], F32, tag="mm", bufs=2)
                for c2 in range(2):
                    nc.tensor.matmul(
                        po[:],
                        ctxT[:, c2, sc * P : (sc + 1) * P],
                        wo_sb[:, c2, mh * QT : (mh + 1) * QT],
                        start=(c2 == 0),
                        stop=(c2 == 1),
                    )
                ob = stage.tile([P, QT], F32, tag="ob", bufs=3)
                nc.scalar.copy(ob[:], po[:])
                nc.sync.dma_start(
                    out[sc * P : (sc + 1) * P, mh * QT : (mh + 1) * QT], ob[:]
                )

    nc.finalize()
    return nc


_NC = None


def _get_nc():
    global _NC
    if _NC is None:
        _NC = build_nc()
    return _NC


def _bf16(a):
    import ml_dtypes

    return np.ascontiguousarray(np.asarray(a, dtype=np.float32)).astype(
        ml_dtypes.bfloat16
    )


def make_in_maps(X_emb, W_q, W_k, W_v, W_out):
    in_maps = []
    for c in range(8):
        b, g = c // 4, c % 4
        cols = slice(g * HC, (g + 1) * HC)
        in_maps.append(
            {
                "x": _bf16(X_emb[b]),
                "wq": _bf16(W_q[:, cols]),
                "wk": _bf16(W_k[:, cols]),
                "wv": _bf16(W_v[:, cols]),
                "wo": _bf16(W_out[cols, :]),
            }
        )
    return in_maps


def combine_outputs(outs):
    return np.stack(
        [
            outs[0] + outs[1] + outs[2] + outs[3],
            outs[4] + outs[5] + outs[6] + outs[7],
        ]
    ).astype(np.float32)


def kernel(X_emb, W_q, W_k, W_v, W_out, _spmd_kwargs=None):
    X_emb = np.asarray(X_emb, dtype=np.float32)
    nc = _get_nc()
    in_maps = make_in_maps(X_emb, W_q, W_k, W_v, W_out)
    res = run_bass_kernel_spmd(
        nc, in_maps, core_ids=list(range(8)), **(_spmd_kwargs or {})
    )
    outs = [res.results[c]["out"] for c in range(8)]
    full = combine_outputs(outs)
    if _spmd_kwargs:
        kernel.last_result = res
    return full
